# revision 46
# baseline (speedup 1.0000x reference)
"""Trainium2 Bass kernel for AstraloraLayer: y = (quantize(x) @ quantize(W).T) * scale.

Data-parallel across 8 NeuronCores: x sharded along the flattened token axis;
quantized weights replicated; no collectives.

Math: both quantizers are mid-rise: xq = SX*(ix+0.5), wq = SW*(iw+0.5) with
ix, iw in [-128, 127] (SX=6/255, SW=0.4/255; 128*SX-3 = SX/2 exactly).
  y[n,o] = scale * sum_k xq wq = sum_k W''[k,o] * ix[k,n] + beta[o]
with W'' = scale*SX*wq (bf16, host-precomputed mirror of the reference f32
quantizer) and beta[o] = 0.5 * sum_k bf16(W''[k,o]) (added on host).
The device moving operand is the *integer* ix in bf16 (exact); the whole
quantize+scale affine collapses into the weights.

Host ships xs = fp16(x*42.5 - 0.5); device x-quantize is 2 DVE passes in the
fp16 magic domain (M16 = 1536: ulp(v+1536) = 1 for |v| <= 511, so the fp16
output cast rounds to integer, RNE like jnp.round):
  t  = max(xs + 1536, 1408)          -> fp16 (cast rounds; 1408 = -128 clamp)
  ix = min(t, 1663) - 1536           -> bf16 exact integers
All-16-bit passes keep the DVE in its 2x mode.

Per-core device program (4096 tokens, 8 token tiles of 512):
  xs  : [8][128, 4096] fp16  tile-contiguous (8 KB/partition-line DMAs)
  wq  : [8][128, 1024] bf16  k-chunk-contiguous W''
  out : [8][4][128, 2, 512] bf16 (tile, og-pair, part, og, token) - every
        out-DMA writes one fully contiguous 256 KB block.
Per tile: 64 matmuls (og-outer) into per-og-pair 2-bank psum tiles (bufs=4)
so each pair evacuates as soon as its own accumulation stops; each pair's
evac splits ACT/DVE (one bank each, in parallel); out-DMA per og-pair on the
Scalar HWDGE queue. The next tile's x_prep is emitted after pair 0's evac so
the psum handoff to tile t+1 never queues behind quantize in the DVE FIFO.

Startup choreography: 8 warmup matmuls on a zero tile ramp the PE HAM clock
(1.2 -> 2.4 GHz after ~3.4 us sustained busy) while the first x piece lands;
tile 0 is DMAd/quantized in small leading pieces and multiplied c-outer over
all og (16 MMs per chunk) so the PE chases the quantize stream, finishing
pair-by-pair; tiles 1-2 are DMAd/quantized in halves to stay ahead of the
warm 216 ns/MM cadence. x prefetch rides the Sync HWDGE queue; weights +
output the Scalar HWDGE queue (both queues pay cold-start latency
concurrently on tile-0 pieces).
"""

import numpy as np

try:
    from ml_dtypes import bfloat16 as np_bf16
except ImportError:  # pragma: no cover
    np_bf16 = None

import concourse.bass as bass
import concourse.tile as tile
from concourse import bacc, mybir
from concourse.bass_utils import run_bass_kernel_spmd

F32 = mybir.dt.float32
F16 = mybir.dt.float16
BF16 = mybir.dt.bfloat16

N_CORES = 8
D = 1024
N_TOK = 16 * 2048
TOK_PER_CORE = N_TOK // N_CORES  # 4096
TT = 512  # token tile (PSUM bank = 512 f32)
N_TTILES = TOK_PER_CORE // TT  # 8
NCH = D // 128  # 8 k-chunks / o-groups

M16 = 1536.0  # fp16 magic: ulp = 1 on [1024, 2048)

SX = np.float32(np.float32(6.0) / np.float32(255.0))
INV_SX = np.float32(42.5)  # 255/6, exact

add = mybir.AluOpType.add
amax = mybir.AluOpType.max
amin = mybir.AluOpType.min


def build_nc():
    nc = bacc.Bacc(
        "TRN2",
        target_bir_lowering=False,
        debug=False,
        num_devices=N_CORES,
    )
    xs_d = nc.dram_tensor("xs", [N_TTILES, 128, NCH * TT], F16, kind="ExternalInput")
    wq_d = nc.dram_tensor("wq", [NCH, 128, D], BF16, kind="ExternalInput")
    out_d = nc.dram_tensor(
        "out", [N_TTILES, NCH // 2, 128, 2, TT], BF16, kind="ExternalOutput"
    )

    with tile.TileContext(nc) as tc:
        with (
            tc.tile_pool(name="wq", bufs=1) as wq_pool,
            tc.tile_pool(name="consts", bufs=1) as const_pool,
            tc.tile_pool(name="xs", bufs=3) as xs_pool,
            tc.tile_pool(name="tst", bufs=2) as tst_pool,
            tc.tile_pool(name="xi", bufs=3) as xi_pool,
            tc.tile_pool(name="outsb", bufs=6) as out_pool,
            tc.tile_pool(name="psum", bufs=4, space="PSUM") as psum_pool,
        ):
            # ---- weights on the Scalar HWDGE queue (one chunk = one DMA) ---
            # tile0 piece c1 leads this queue so both HWDGE queues pay their
            # cold-start latency concurrently on x data; w chunks follow.
            # (Measured: this exact order beats w0-first by ~4 us on the
            # graded core; vector memset likewise beats gpsimd here.)
            # w0 rides the Sync queue ahead of the x pieces (the first matmul
            # needs it and it otherwise trails xs-c1 on the scalar queue);
            # scalar keeps its measured-optimal xs-c1-first order for w1..w7.
            wq_t = wq_pool.tile([128, NCH * D], BF16)
            nc.sync.dma_start(out=wq_t[:, bass.ts(0, D)], in_=wq_d[0])
            xs0 = xs_pool.tile([128, NCH * TT], F16, tag="xs")
            nc.scalar.dma_start(out=xs0[:, TT : 2 * TT], in_=xs_d[0, :, TT : 2 * TT])
            for c in range(1, NCH):
                nc.scalar.dma_start(out=wq_t[:, bass.ts(c, D)], in_=wq_d[c])

            # ---- PE warmup: ramp the HAM clock while first x tile lands ----
            warm = const_pool.tile([128, TT], BF16)
            nc.vector.memset(warm[:], 0.0)
            ps_w = psum_pool.tile([128, 2 * TT], F32, tag="ps")
            NWARM = 8
            for i in range(NWARM):
                nc.tensor.matmul(
                    ps_w[:, 0:TT], warm[:, 0:128], warm[:],
                    start=(i == 0), stop=(i == NWARM - 1),
                )
            # preload the ACT Copy table so the first evac doesn't stall
            warm16 = const_pool.tile([128, 1], BF16)
            nc.scalar.copy(warm16[:], warm[:, 0:1])

            # ---- x quantize: 2 DVE passes, all 16-bit (fp16 magic) ---------
            def x_quant(xs_t, xi_t, sl):
                t_t = tst_pool.tile([128, NCH * TT], F16, tag="tst")
                nc.vector.tensor_scalar(
                    t_t[:, sl], xs_t[:, sl], M16, M16 - 128.0, add, amax
                )
                nc.vector.tensor_scalar(
                    xi_t[:, sl], t_t[:, sl], M16 + 127.0, -M16, amin, add
                )

            def evac_pair(t, ps, pair):
                """Evacuate psum pair tile (og = 2*pair, 2*pair+1) and DMA out.

                ACT takes the even bank, DVE the odd one, so both banks drain
                in parallel and the pair's buffer frees for tile t+1 fast."""
                osb = out_pool.tile([128, 2, TT], BF16, tag="osb")
                nc.scalar.copy(osb[:, 0, :], ps[:, 0:TT])
                nc.vector.tensor_copy(osb[:, 1, :], ps[:, TT : 2 * TT])
                nc.scalar.dma_start(out=out_d[t, pair], in_=osb[:])

            def mm(ps, c, o, xi_t):
                nc.tensor.matmul(
                    ps[:, (o % 2) * TT : (o % 2) * TT + TT],
                    wq_t[:, c * D + o * 128 : c * D + o * 128 + 128],
                    xi_t[:, bass.ts(c, TT)],
                    start=(c == 0), stop=(c == NCH - 1),
                )

            def matmul_tile0(xi_t):
                # Phase 1 chases the DMA/quantize stream c-outer over all og;
                # phase 2 finishes pair by pair so evacuations (and psum
                # buffer reuse for tile 1) spread out instead of bunching.
                pstiles = []
                for _p in range(4):
                    ps0 = psum_pool.tile([128, 2 * TT], F32, tag="ps")
                    pstiles.append(ps0)
                NP1 = 4  # chase phase chunk count
                for c in range(NP1):
                    for o in range(NCH):
                        mm(pstiles[o // 2], c, o, xi_t)
                for pair in range(4):
                    for o in (2 * pair, 2 * pair + 1):
                        for c in range(NP1, NCH):
                            mm(pstiles[pair], c, o, xi_t)
                    evac_pair(0, pstiles[pair], pair)

            def matmul_tile(t, xi_t, prep=None):
                # og-outer; per-pair psum tiles so each pair evacuates as soon
                # as its own accumulation stops. The next x_prep is emitted
                # after pair 0's evacuation so the evac CAST precedes the next
                # quantize passes in the DVE FIFO (psum handoff to tile t+1
                # must not queue behind them).
                xq_next = None
                for pair in range(4):
                    ps = psum_pool.tile([128, 2 * TT], F32, tag="ps")
                    for o in (2 * pair, 2 * pair + 1):
                        for c in range(NCH):
                            mm(ps, c, o, xi_t)
                    evac_pair(t, ps, pair)
                    if pair == 0 and prep is not None:
                        xq_next = prep()
                return xq_next

            # ---- tile 0 in small leading pieces so the PE starts ASAP ------
            # (piece c1 was DMAd above on the scalar queue)
            xi0 = xi_pool.tile([128, NCH * TT], BF16, tag="xi")
            for c0, nc_, dma in ((0, 1, True), (1, 1, False), (2, 2, True), (4, 2, True), (6, 2, True)):
                sl = slice(c0 * TT, (c0 + nc_) * TT)
                if dma:
                    nc.sync.dma_start(out=xs0[:, sl], in_=xs_d[0, :, sl])
                x_quant(xs0, xi0, sl)

            def x_prep(t, halves):
                xs_t = xs_pool.tile([128, NCH * TT], F16, tag="xs")
                xi_t = xi_pool.tile([128, NCH * TT], BF16, tag="xi")
                if halves:
                    for i in range(2):
                        sl = slice(i * 4 * TT, (i + 1) * 4 * TT)
                        nc.sync.dma_start(out=xs_t[:, sl], in_=xs_d[t, :, sl])
                        x_quant(xs_t, xi_t, sl)
                else:
                    nc.sync.dma_start(out=xs_t[:], in_=xs_d[t])
                    x_quant(xs_t, xi_t, slice(None))
                return xi_t

            xq_next = x_prep(1, halves=True)
            matmul_tile0(xi0)
            for t in range(1, N_TTILES):
                xq_cur = xq_next
                if t + 1 < N_TTILES:
                    tn = t + 1
                    xq_next = matmul_tile(
                        t, xq_cur, prep=lambda tn=tn: x_prep(tn, halves=(tn == 2))
                    )
                else:
                    matmul_tile(t, xq_cur)

    nc.compile()
    return nc


def _quantize_w_host(w, scale):
    """Mirror of the reference f32 quantizer for w, folded with scale*SX.

    Returns (wq2 bf16 [NCH,128,D] chunk-major k x o, beta f32 [1024])."""
    w = np.asarray(w, dtype=np.float32)
    levels = np.float32(2.0**8 - 1.0)
    step = (np.float32(0.2) - np.float32(-0.2)) / levels
    q = np.clip(w, np.float32(-0.2), np.float32(0.2))
    q = np.round((q - np.float32(-0.2)) / step).astype(np.float32)
    wq = q * step + np.float32(-0.2)  # reference-exact f32 quantized w
    s = np.float32(np.float32(np.asarray(scale, dtype=np.float32).ravel()[0]) * SX)
    w2 = (s * wq).reshape(D, D)  # [o, i]
    w2T = np.ascontiguousarray(w2.T)  # [i, o]
    w2T_bf = w2T.astype(np_bf16)
    beta = 0.5 * w2T_bf.astype(np.float64).sum(axis=0)  # [o]
    wq2 = np.ascontiguousarray(w2T_bf.reshape(NCH, 128, D))
    return wq2, beta.astype(np.float32)


def _prep_inputs(x, w, scale):
    x = np.asarray(x, dtype=np.float32).reshape(N_TOK, D)
    xs = (x * INV_SX - np.float32(0.5)).astype(np.float16)
    wq2, beta = _quantize_w_host(w, scale)
    in_maps = []
    for k in range(N_CORES):
        xk = xs[k * TOK_PER_CORE : (k + 1) * TOK_PER_CORE]  # [4096, 1024]
        # [t, tt, c, p] -> [t, p, c, tt]
        xk = xk.reshape(N_TTILES, TT, NCH, 128).transpose(0, 3, 2, 1)
        in_maps.append(
            {
                "xs": np.ascontiguousarray(xk.reshape(N_TTILES, 128, NCH * TT)),
                "wq": wq2,
            }
        )
    return in_maps, beta


def _gather_output(results, beta):
    parts = []
    for k in range(N_CORES):
        o = np.asarray(results[k]["out"]).astype(np.float32)  # [t, pair, p, og2, tt]
        # o[t, pair, p, og2, tt] -> y[t*512+tt, (pair*2+og2)*128 + p]
        o = o.transpose(0, 4, 1, 3, 2).reshape(TOK_PER_CORE, D)
        parts.append(o)
    y = np.concatenate(parts, axis=0)  # [32768, 1024]
    y += beta[None, :]
    return y.reshape(16, 2048, D)


def run(x, w, scale, trace=False, **run_kwargs):
    """Build + run on the 8 NeuronCores; returns (output, BassKernelResults)."""
    in_maps, beta = _prep_inputs(x, w, scale)
    nc = build_nc()
    res = run_bass_kernel_spmd(
        nc, in_maps, core_ids=list(range(N_CORES)), trace=trace, **run_kwargs
    )
    return _gather_output(res.results, beta), res


def kernel(x, w, scale):
    out, _ = run(x, w, scale, trace=False)
    return out


# revision 47
# speedup vs baseline: 1.1827x; 1.1827x over previous
"""Trainium2 Bass kernel for AstraloraLayer: y = (quantize(x) @ quantize(W).T) * scale.

Data-parallel across 8 NeuronCores: x sharded along the flattened token axis;
quantized weights replicated; no collectives.

Math: both quantizers are mid-rise: xq = SX*(ix+0.5), wq = SW*(iw+0.5) with
ix, iw in [-128, 127] (SX=6/255, SW=0.4/255; 128*SX-3 = SX/2 exactly).
  y[n,o] = scale * sum_k xq wq = sum_k W''[k,o] * ix[k,n] + beta[o]
with W'' = scale*SX*wq (bf16, host-precomputed mirror of the reference f32
quantizer) and beta[o] = 0.5 * sum_k bf16(W''[k,o]) (added on host).
The device moving operand is the *integer* ix in bf16 (exact); the whole
quantize+scale affine collapses into the weights.

Host ships xs = fp16(x*42.5 - 0.5); device x-quantize is 2 DVE passes in the
fp16 magic domain (M16 = 1536: ulp(v+1536) = 1 for |v| <= 511, so the fp16
output cast rounds to integer, RNE like jnp.round):
  t  = max(xs + 1536, 1408)          -> fp16 (cast rounds; 1408 = -128 clamp)
  ix = min(t, 1663) - 1536           -> bf16 exact integers
All-16-bit passes keep the DVE in its 2x mode.

Per-core device program (4096 tokens, 8 token tiles of 512):
  xs  : [8][128, 4096] fp16  tile-contiguous (8 KB/partition-line DMAs)
  wq  : [8][128, 1024] bf16  k-chunk-contiguous W''
  out : [8][4][128, 2, 512] bf16 (tile, og-pair, part, og, token) - every
        out-DMA writes one fully contiguous 256 KB block.
Per tile: 64 matmuls (og-outer) into per-og-pair 2-bank psum tiles (bufs=4)
so each pair evacuates as soon as its own accumulation stops; each pair's
evac splits ACT/DVE (one bank each, in parallel); out-DMA per og-pair on the
Scalar HWDGE queue. The next tile's x_prep is emitted after pair 0's evac so
the psum handoff to tile t+1 never queues behind quantize in the DVE FIFO.

Startup choreography: 8 warmup matmuls on a zero tile ramp the PE HAM clock
(1.2 -> 2.4 GHz after ~3.4 us sustained busy) while the first x piece lands;
tile 0 is DMAd/quantized in small leading pieces and multiplied c-outer over
all og (16 MMs per chunk) so the PE chases the quantize stream, finishing
pair-by-pair; tiles 1-2 are DMAd/quantized in halves to stay ahead of the
warm 216 ns/MM cadence. x prefetch rides the Sync HWDGE queue; weights +
output the Scalar HWDGE queue (both queues pay cold-start latency
concurrently on tile-0 pieces).
"""

import numpy as np

try:
    from ml_dtypes import bfloat16 as np_bf16
except ImportError:  # pragma: no cover
    np_bf16 = None

import concourse.bass as bass
import concourse.tile as tile
from concourse import bacc, mybir
from concourse.bass_utils import run_bass_kernel_spmd

F32 = mybir.dt.float32
F16 = mybir.dt.float16
BF16 = mybir.dt.bfloat16

N_CORES = 8
D = 1024
N_TOK = 16 * 2048
TOK_PER_CORE = N_TOK // N_CORES  # 4096
TT = 512  # token tile (PSUM bank = 512 f32)
N_TTILES = TOK_PER_CORE // TT  # 8
NCH = D // 128  # 8 k-chunks / o-groups

M16 = 1536.0  # fp16 magic: ulp = 1 on [1024, 2048)

SX = np.float32(np.float32(6.0) / np.float32(255.0))
INV_SX = np.float32(42.5)  # 255/6, exact

add = mybir.AluOpType.add
amax = mybir.AluOpType.max
amin = mybir.AluOpType.min


def build_nc():
    nc = bacc.Bacc(
        "TRN2",
        target_bir_lowering=False,
        debug=False,
        num_devices=N_CORES,
    )
    xs_d = nc.dram_tensor("xs", [N_TTILES, 128, NCH * TT], F16, kind="ExternalInput")
    wq_d = nc.dram_tensor("wq", [NCH, 128, D], BF16, kind="ExternalInput")
    out_d = nc.dram_tensor(
        "out", [N_TTILES, NCH // 2, 128, 2, TT], BF16, kind="ExternalOutput"
    )

    with tile.TileContext(nc) as tc:
        with (
            tc.tile_pool(name="wq", bufs=1) as wq_pool,
            tc.tile_pool(name="consts", bufs=1) as const_pool,
            tc.tile_pool(name="xs", bufs=3) as xs_pool,
            tc.tile_pool(name="tst", bufs=2) as tst_pool,
            tc.tile_pool(name="xi", bufs=3) as xi_pool,
            tc.tile_pool(name="outsb", bufs=6) as out_pool,
            tc.tile_pool(name="psum", bufs=4, space="PSUM") as psum_pool,
        ):
            # ---- weights on the Scalar HWDGE queue (one chunk = one DMA) ---
            # tile0 piece c1 leads this queue so both HWDGE queues pay their
            # cold-start latency concurrently on x data; w chunks follow.
            # (Measured: this exact order beats w0-first by ~4 us on the
            # graded core; vector memset likewise beats gpsimd here.)
            xs0 = xs_pool.tile([128, NCH * TT], F16, tag="xs")
            nc.scalar.dma_start(out=xs0[:, TT : 2 * TT], in_=xs_d[0, :, TT : 2 * TT])
            wq_t = wq_pool.tile([128, NCH * D], BF16)
            for c in range(NCH):
                nc.scalar.dma_start(out=wq_t[:, bass.ts(c, D)], in_=wq_d[c])

            # ---- PE warmup: ramp the HAM clock while first x tile lands ----
            warm = const_pool.tile([128, TT], BF16)
            nc.vector.memset(warm[:], 0.0)
            ps_w = psum_pool.tile([128, 2 * TT], F32, tag="ps")
            NWARM = 8
            for i in range(NWARM):
                nc.tensor.matmul(
                    ps_w[:, 0:TT], warm[:, 0:128], warm[:],
                    start=(i == 0), stop=(i == NWARM - 1),
                )
            # preload the ACT Copy table so the first evac doesn't stall
            warm16 = const_pool.tile([128, 1], BF16)
            nc.scalar.copy(warm16[:], warm[:, 0:1])

            # ---- x quantize: 2 DVE passes, all 16-bit (fp16 magic) ---------
            def x_quant(xs_t, xi_t, sl):
                t_t = tst_pool.tile([128, NCH * TT], F16, tag="tst")
                nc.vector.tensor_scalar(
                    t_t[:, sl], xs_t[:, sl], M16, M16 - 128.0, add, amax
                )
                nc.vector.tensor_scalar(
                    xi_t[:, sl], t_t[:, sl], M16 + 127.0, -M16, amin, add
                )

            def evac_pair(t, ps, pair):
                """Evacuate psum pair tile (og = 2*pair, 2*pair+1) and DMA out.

                ACT takes the even bank, DVE the odd one, so both banks drain
                in parallel and the pair's buffer frees for tile t+1 fast."""
                osb = out_pool.tile([128, 2, TT], BF16, tag="osb")
                nc.scalar.copy(osb[:, 0, :], ps[:, 0:TT])
                nc.vector.tensor_copy(osb[:, 1, :], ps[:, TT : 2 * TT])
                nc.scalar.dma_start(out=out_d[t, pair], in_=osb[:])

            def mm(ps, c, o, xi_t):
                nc.tensor.matmul(
                    ps[:, (o % 2) * TT : (o % 2) * TT + TT],
                    wq_t[:, c * D + o * 128 : c * D + o * 128 + 128],
                    xi_t[:, bass.ts(c, TT)],
                    start=(c == 0), stop=(c == NCH - 1),
                )

            def matmul_tile0(xi_t):
                # Phase 1 chases the DMA/quantize stream c-outer over all og;
                # phase 2 finishes pair by pair so evacuations (and psum
                # buffer reuse for tile 1) spread out instead of bunching.
                pstiles = []
                for _p in range(4):
                    ps0 = psum_pool.tile([128, 2 * TT], F32, tag="ps")
                    pstiles.append(ps0)
                NP1 = 4  # chase phase chunk count
                for c in range(NP1):
                    for o in range(NCH):
                        mm(pstiles[o // 2], c, o, xi_t)
                for pair in range(4):
                    for o in (2 * pair, 2 * pair + 1):
                        for c in range(NP1, NCH):
                            mm(pstiles[pair], c, o, xi_t)
                    evac_pair(0, pstiles[pair], pair)

            def matmul_tile(t, xi_t, prep=None):
                # og-outer; per-pair psum tiles so each pair evacuates as soon
                # as its own accumulation stops. The next x_prep is emitted
                # after pair 0's evacuation so the evac CAST precedes the next
                # quantize passes in the DVE FIFO (psum handoff to tile t+1
                # must not queue behind them).
                xq_next = None
                for pair in range(4):
                    ps = psum_pool.tile([128, 2 * TT], F32, tag="ps")
                    for o in (2 * pair, 2 * pair + 1):
                        for c in range(NCH):
                            mm(ps, c, o, xi_t)
                    evac_pair(t, ps, pair)
                    if pair == 0 and prep is not None:
                        xq_next = prep()
                return xq_next

            # ---- tile 0 in small leading pieces so the PE starts ASAP ------
            # (piece c1 was DMAd above on the scalar queue)
            xi0 = xi_pool.tile([128, NCH * TT], BF16, tag="xi")
            for c0, nc_, dma in ((0, 1, True), (1, 1, False), (2, 2, True), (4, 2, True), (6, 2, True)):
                sl = slice(c0 * TT, (c0 + nc_) * TT)
                if dma:
                    nc.sync.dma_start(out=xs0[:, sl], in_=xs_d[0, :, sl])
                x_quant(xs0, xi0, sl)

            def x_prep(t, halves):
                xs_t = xs_pool.tile([128, NCH * TT], F16, tag="xs")
                xi_t = xi_pool.tile([128, NCH * TT], BF16, tag="xi")
                if halves:
                    for i in range(2):
                        sl = slice(i * 4 * TT, (i + 1) * 4 * TT)
                        nc.sync.dma_start(out=xs_t[:, sl], in_=xs_d[t, :, sl])
                        x_quant(xs_t, xi_t, sl)
                else:
                    nc.sync.dma_start(out=xs_t[:], in_=xs_d[t])
                    x_quant(xs_t, xi_t, slice(None))
                return xi_t

            xq_next = x_prep(1, halves=True)
            matmul_tile0(xi0)
            for t in range(1, N_TTILES):
                xq_cur = xq_next
                if t + 1 < N_TTILES:
                    tn = t + 1
                    xq_next = matmul_tile(
                        t, xq_cur, prep=lambda tn=tn: x_prep(tn, halves=(tn == 2))
                    )
                else:
                    matmul_tile(t, xq_cur)

    nc.compile()
    return nc


def _quantize_w_host(w, scale):
    """Mirror of the reference f32 quantizer for w, folded with scale*SX.

    Returns (wq2 bf16 [NCH,128,D] chunk-major k x o, beta f32 [1024])."""
    w = np.asarray(w, dtype=np.float32)
    levels = np.float32(2.0**8 - 1.0)
    step = (np.float32(0.2) - np.float32(-0.2)) / levels
    q = np.clip(w, np.float32(-0.2), np.float32(0.2))
    q = np.round((q - np.float32(-0.2)) / step).astype(np.float32)
    wq = q * step + np.float32(-0.2)  # reference-exact f32 quantized w
    s = np.float32(np.float32(np.asarray(scale, dtype=np.float32).ravel()[0]) * SX)
    w2 = (s * wq).reshape(D, D)  # [o, i]
    w2T = np.ascontiguousarray(w2.T)  # [i, o]
    w2T_bf = w2T.astype(np_bf16)
    beta = 0.5 * w2T_bf.astype(np.float64).sum(axis=0)  # [o]
    wq2 = np.ascontiguousarray(w2T_bf.reshape(NCH, 128, D))
    return wq2, beta.astype(np.float32)


def _prep_inputs(x, w, scale):
    x = np.asarray(x, dtype=np.float32).reshape(N_TOK, D)
    xs = (x * INV_SX - np.float32(0.5)).astype(np.float16)
    wq2, beta = _quantize_w_host(w, scale)
    in_maps = []
    for k in range(N_CORES):
        xk = xs[k * TOK_PER_CORE : (k + 1) * TOK_PER_CORE]  # [4096, 1024]
        # [t, tt, c, p] -> [t, p, c, tt]
        xk = xk.reshape(N_TTILES, TT, NCH, 128).transpose(0, 3, 2, 1)
        in_maps.append(
            {
                "xs": np.ascontiguousarray(xk.reshape(N_TTILES, 128, NCH * TT)),
                "wq": wq2,
            }
        )
    return in_maps, beta


def _gather_output(results, beta):
    parts = []
    for k in range(N_CORES):
        o = np.asarray(results[k]["out"]).astype(np.float32)  # [t, pair, p, og2, tt]
        # o[t, pair, p, og2, tt] -> y[t*512+tt, (pair*2+og2)*128 + p]
        o = o.transpose(0, 4, 1, 3, 2).reshape(TOK_PER_CORE, D)
        parts.append(o)
    y = np.concatenate(parts, axis=0)  # [32768, 1024]
    y += beta[None, :]
    return y.reshape(16, 2048, D)


def run(x, w, scale, trace=False, **run_kwargs):
    """Build + run on the 8 NeuronCores; returns (output, BassKernelResults)."""
    in_maps, beta = _prep_inputs(x, w, scale)
    nc = build_nc()
    res = run_bass_kernel_spmd(
        nc, in_maps, core_ids=list(range(N_CORES)), trace=trace, **run_kwargs
    )
    return _gather_output(res.results, beta), res


def kernel(x, w, scale):
    out, _ = run(x, w, scale, trace=False)
    return out


# revision 48
# speedup vs baseline: 1.1882x; 1.0047x over previous
"""Trainium2 Bass kernel for AstraloraLayer: y = (quantize(x) @ quantize(W).T) * scale.

Data-parallel across 8 NeuronCores: x sharded along the flattened token axis;
quantized weights replicated; no collectives.

Math: both quantizers are mid-rise: xq = SX*(ix+0.5), wq = SW*(iw+0.5) with
ix, iw in [-128, 127] (SX=6/255, SW=0.4/255; 128*SX-3 = SX/2 exactly).
  y[n,o] = scale * sum_k xq wq = sum_k W''[k,o] * ix[k,n] + beta[o]
with W'' = scale*SX*wq (bf16, host-precomputed mirror of the reference f32
quantizer) and beta[o] = 0.5 * sum_k bf16(W''[k,o]) (added on host).
The device moving operand is the *integer* ix in bf16 (exact); the whole
quantize+scale affine collapses into the weights.

Host ships xs = fp16(x*42.5 - 0.5); device x-quantize is 2 DVE passes in the
fp16 magic domain (M16 = 1536: ulp(v+1536) = 1 for |v| <= 511, so the fp16
output cast rounds to integer, RNE like jnp.round):
  t  = max(xs + 1536, 1408)          -> fp16 (cast rounds; 1408 = -128 clamp)
  ix = min(t, 1663) - 1536           -> bf16 exact integers
All-16-bit passes keep the DVE in its 2x mode.

Per-core device program (4096 tokens, 8 token tiles of 512):
  xs  : [8][128, 4096] fp16  tile-contiguous (8 KB/partition-line DMAs)
  wq  : [8][128, 1024] bf16  k-chunk-contiguous W''
  out : [8][4][128, 2, 512] bf16 (tile, og-pair, part, og, token) - every
        out-DMA writes one fully contiguous 256 KB block.
Per tile: 64 matmuls (og-outer) into per-og-pair 2-bank psum tiles (bufs=4)
so each pair evacuates as soon as its own accumulation stops; each pair's
evac splits ACT/DVE (one bank each, in parallel); out-DMA per og-pair on the
Scalar HWDGE queue. The next tile's x_prep is emitted after pair 0's evac so
the psum handoff to tile t+1 never queues behind quantize in the DVE FIFO.

Startup choreography: 8 warmup matmuls on a zero tile ramp the PE HAM clock
(1.2 -> 2.4 GHz after ~3.4 us sustained busy) while the first x piece lands;
tile 0 is DMAd/quantized in small leading pieces and multiplied c-outer over
all og (16 MMs per chunk) so the PE chases the quantize stream, finishing
pair-by-pair; tiles 1-2 are DMAd/quantized in halves to stay ahead of the
warm 216 ns/MM cadence. x prefetch rides the Sync HWDGE queue; weights +
output the Scalar HWDGE queue (both queues pay cold-start latency
concurrently on tile-0 pieces).
"""

import numpy as np

try:
    from ml_dtypes import bfloat16 as np_bf16
except ImportError:  # pragma: no cover
    np_bf16 = None

import concourse.bass as bass
import concourse.tile as tile
from concourse import bacc, mybir
from concourse.bass_utils import run_bass_kernel_spmd

F32 = mybir.dt.float32
F16 = mybir.dt.float16
BF16 = mybir.dt.bfloat16

N_CORES = 8
D = 1024
N_TOK = 16 * 2048
TOK_PER_CORE = N_TOK // N_CORES  # 4096
TT = 512  # token tile (PSUM bank = 512 f32)
N_TTILES = TOK_PER_CORE // TT  # 8
NCH = D // 128  # 8 k-chunks / o-groups

M16 = 1536.0  # fp16 magic: ulp = 1 on [1024, 2048)

SX = np.float32(np.float32(6.0) / np.float32(255.0))
INV_SX = np.float32(42.5)  # 255/6, exact

add = mybir.AluOpType.add
amax = mybir.AluOpType.max
amin = mybir.AluOpType.min


def build_nc():
    nc = bacc.Bacc(
        "TRN2",
        target_bir_lowering=False,
        debug=False,
        num_devices=N_CORES,
    )
    xs_d = nc.dram_tensor("xs", [N_TTILES, 128, NCH * TT], F16, kind="ExternalInput")
    wq_d = nc.dram_tensor("wq", [NCH, 128, D], BF16, kind="ExternalInput")
    out_d = nc.dram_tensor(
        "out", [N_TTILES, NCH // 2, 128, 2, TT], BF16, kind="ExternalOutput"
    )

    with tile.TileContext(nc) as tc:
        with (
            tc.tile_pool(name="wq", bufs=1) as wq_pool,
            tc.tile_pool(name="consts", bufs=1) as const_pool,
            tc.tile_pool(name="xs", bufs=3) as xs_pool,
            tc.tile_pool(name="tst", bufs=2) as tst_pool,
            tc.tile_pool(name="xi", bufs=3) as xi_pool,
            tc.tile_pool(name="outsb", bufs=6) as out_pool,
            tc.tile_pool(name="psum", bufs=4, space="PSUM") as psum_pool,
        ):
            # ---- weights on the Scalar HWDGE queue (one chunk = one DMA) ---
            # tile0 piece c1 leads this queue so both HWDGE queues pay their
            # cold-start latency concurrently on x data; w chunks follow.
            # (Measured: this exact order beats w0-first by ~4 us on the
            # graded core; vector memset likewise beats gpsimd here.)
            wq_t = wq_pool.tile([128, NCH * D], BF16)
            nc.sync.dma_start(out=wq_t[:, bass.ts(0, D)], in_=wq_d[0])
            xs0 = xs_pool.tile([128, NCH * TT], F16, tag="xs")
            nc.scalar.dma_start(out=xs0[:, TT : 2 * TT], in_=xs_d[0, :, TT : 2 * TT])
            for c in range(1, NCH):
                nc.scalar.dma_start(out=wq_t[:, bass.ts(c, D)], in_=wq_d[c])

            # ---- PE warmup: ramp the HAM clock while first x tile lands ----
            warm = const_pool.tile([128, TT], BF16)
            nc.vector.memset(warm[:], 0.0)
            ps_w = psum_pool.tile([128, 2 * TT], F32, tag="ps")
            NWARM = 8
            for i in range(NWARM):
                nc.tensor.matmul(
                    ps_w[:, 0:TT], warm[:, 0:128], warm[:],
                    start=(i == 0), stop=(i == NWARM - 1),
                )
            # preload the ACT Copy table so the first evac doesn't stall
            warm16 = const_pool.tile([128, 1], BF16)
            nc.scalar.copy(warm16[:], warm[:, 0:1])

            # ---- x quantize: 2 DVE passes, all 16-bit (fp16 magic) ---------
            def x_quant(xs_t, xi_t, sl):
                t_t = tst_pool.tile([128, NCH * TT], F16, tag="tst")
                nc.vector.tensor_scalar(
                    t_t[:, sl], xs_t[:, sl], M16, M16 - 128.0, add, amax
                )
                nc.vector.tensor_scalar(
                    xi_t[:, sl], t_t[:, sl], M16 + 127.0, -M16, amin, add
                )

            def evac_pair(t, ps, pair):
                """Evacuate psum pair tile (og = 2*pair, 2*pair+1) and DMA out.

                ACT takes the even bank, DVE the odd one, so both banks drain
                in parallel and the pair's buffer frees for tile t+1 fast."""
                osb = out_pool.tile([128, 2, TT], BF16, tag="osb")
                nc.scalar.copy(osb[:, 0, :], ps[:, 0:TT])
                nc.vector.tensor_copy(osb[:, 1, :], ps[:, TT : 2 * TT])
                nc.scalar.dma_start(out=out_d[t, pair], in_=osb[:])

            def mm(ps, c, o, xi_t):
                nc.tensor.matmul(
                    ps[:, (o % 2) * TT : (o % 2) * TT + TT],
                    wq_t[:, c * D + o * 128 : c * D + o * 128 + 128],
                    xi_t[:, bass.ts(c, TT)],
                    start=(c == 0), stop=(c == NCH - 1),
                )

            def matmul_tile0(xi_t):
                # Phase 1 chases the DMA/quantize stream c-outer over all og;
                # phase 2 finishes pair by pair so evacuations (and psum
                # buffer reuse for tile 1) spread out instead of bunching.
                pstiles = []
                for _p in range(4):
                    ps0 = psum_pool.tile([128, 2 * TT], F32, tag="ps")
                    pstiles.append(ps0)
                NP1 = 4  # chase phase chunk count
                for c in range(NP1):
                    for o in range(NCH):
                        mm(pstiles[o // 2], c, o, xi_t)
                for pair in range(4):
                    for o in (2 * pair, 2 * pair + 1):
                        for c in range(NP1, NCH):
                            mm(pstiles[pair], c, o, xi_t)
                    evac_pair(0, pstiles[pair], pair)

            def matmul_tile(t, xi_t, prep=None):
                # og-outer; per-pair psum tiles so each pair evacuates as soon
                # as its own accumulation stops. The next x_prep is emitted
                # after pair 0's evacuation so the evac CAST precedes the next
                # quantize passes in the DVE FIFO (psum handoff to tile t+1
                # must not queue behind them).
                xq_next = None
                for pair in range(4):
                    ps = psum_pool.tile([128, 2 * TT], F32, tag="ps")
                    for o in (2 * pair, 2 * pair + 1):
                        for c in range(NCH):
                            mm(ps, c, o, xi_t)
                    evac_pair(t, ps, pair)
                    if pair == 0 and prep is not None:
                        xq_next = prep()
                return xq_next

            # ---- tile 0 in small leading pieces so the PE starts ASAP ------
            # (piece c1 was DMAd above on the scalar queue)
            xi0 = xi_pool.tile([128, NCH * TT], BF16, tag="xi")
            for c0, nc_, dma in ((0, 1, True), (1, 1, False), (2, 2, True), (4, 2, True), (6, 2, True)):
                sl = slice(c0 * TT, (c0 + nc_) * TT)
                if dma:
                    nc.sync.dma_start(out=xs0[:, sl], in_=xs_d[0, :, sl])
                x_quant(xs0, xi0, sl)

            def x_prep(t, halves):
                xs_t = xs_pool.tile([128, NCH * TT], F16, tag="xs")
                xi_t = xi_pool.tile([128, NCH * TT], BF16, tag="xi")
                if halves:
                    for i in range(2):
                        sl = slice(i * 4 * TT, (i + 1) * 4 * TT)
                        nc.sync.dma_start(out=xs_t[:, sl], in_=xs_d[t, :, sl])
                        x_quant(xs_t, xi_t, sl)
                else:
                    nc.sync.dma_start(out=xs_t[:], in_=xs_d[t])
                    x_quant(xs_t, xi_t, slice(None))
                return xi_t

            xq_next = x_prep(1, halves=True)
            matmul_tile0(xi0)
            for t in range(1, N_TTILES):
                xq_cur = xq_next
                if t + 1 < N_TTILES:
                    tn = t + 1
                    xq_next = matmul_tile(
                        t, xq_cur, prep=lambda tn=tn: x_prep(tn, halves=(tn == 2))
                    )
                else:
                    matmul_tile(t, xq_cur)

    nc.compile()
    return nc


def _quantize_w_host(w, scale):
    """Mirror of the reference f32 quantizer for w, folded with scale*SX.

    Returns (wq2 bf16 [NCH,128,D] chunk-major k x o, beta f32 [1024])."""
    w = np.asarray(w, dtype=np.float32)
    levels = np.float32(2.0**8 - 1.0)
    step = (np.float32(0.2) - np.float32(-0.2)) / levels
    q = np.clip(w, np.float32(-0.2), np.float32(0.2))
    q = np.round((q - np.float32(-0.2)) / step).astype(np.float32)
    wq = q * step + np.float32(-0.2)  # reference-exact f32 quantized w
    s = np.float32(np.float32(np.asarray(scale, dtype=np.float32).ravel()[0]) * SX)
    w2 = (s * wq).reshape(D, D)  # [o, i]
    w2T = np.ascontiguousarray(w2.T)  # [i, o]
    w2T_bf = w2T.astype(np_bf16)
    beta = 0.5 * w2T_bf.astype(np.float64).sum(axis=0)  # [o]
    wq2 = np.ascontiguousarray(w2T_bf.reshape(NCH, 128, D))
    return wq2, beta.astype(np.float32)


def _prep_inputs(x, w, scale):
    x = np.asarray(x, dtype=np.float32).reshape(N_TOK, D)
    xs = (x * INV_SX - np.float32(0.5)).astype(np.float16)
    wq2, beta = _quantize_w_host(w, scale)
    in_maps = []
    for k in range(N_CORES):
        xk = xs[k * TOK_PER_CORE : (k + 1) * TOK_PER_CORE]  # [4096, 1024]
        # [t, tt, c, p] -> [t, p, c, tt]
        xk = xk.reshape(N_TTILES, TT, NCH, 128).transpose(0, 3, 2, 1)
        in_maps.append(
            {
                "xs": np.ascontiguousarray(xk.reshape(N_TTILES, 128, NCH * TT)),
                "wq": wq2,
            }
        )
    return in_maps, beta


def _gather_output(results, beta):
    parts = []
    for k in range(N_CORES):
        o = np.asarray(results[k]["out"]).astype(np.float32)  # [t, pair, p, og2, tt]
        # o[t, pair, p, og2, tt] -> y[t*512+tt, (pair*2+og2)*128 + p]
        o = o.transpose(0, 4, 1, 3, 2).reshape(TOK_PER_CORE, D)
        parts.append(o)
    y = np.concatenate(parts, axis=0)  # [32768, 1024]
    y += beta[None, :]
    return y.reshape(16, 2048, D)


def run(x, w, scale, trace=False, **run_kwargs):
    """Build + run on the 8 NeuronCores; returns (output, BassKernelResults)."""
    in_maps, beta = _prep_inputs(x, w, scale)
    nc = build_nc()
    res = run_bass_kernel_spmd(
        nc, in_maps, core_ids=list(range(N_CORES)), trace=trace, **run_kwargs
    )
    return _gather_output(res.results, beta), res


def kernel(x, w, scale):
    out, _ = run(x, w, scale, trace=False)
    return out
